# revision 2
# baseline (speedup 1.0000x reference)
"""Delay-and-sum beamformer on 8 TRN2 NeuronCores.

Problem: x[16, 100000, 128] f32 -> out[b, t] = mean_s x[b, t + d_s, s],
d_s = round(s * sin(30deg) / 2) in [0, 32] (zero-padded past t = T-1).

Sharding: pure data parallel over batch (2 batches per core).

Per-core layout ("stripe" scheme): for each batch, partition p owns time
rows [784*p, 784*(p+1)) of a zero-padded T_pad = 100352 signal, loaded in
14 chunks of 56 rows through a 3-slot ring as [row, sensor] (row pitch
128 f32).  HBM traffic is exactly the (padded) input + output -- no halo
re-reads and no bulk SBUF copies.

The delay structure groups sensors as {0,1,2} (d=0), {4k-1..4k+2} (d=k,
k=1..31), {127} (d=32).  Compute is two passes:

  1. G-pass (per chunk, while resident): per time row t, 33 group sums
     G[t] = [G0, G1..G31, x127] -- one strided DVE reduce for k=1..31,
     one small DVE reduce for G0, one ACT copy for sensor 127 -- into a
     persistent per-batch G buffer [128, (784+32)*33] f32.
  2. diag-pass: out[tau] = sum_d G[tau+d][d] -- a single strided DVE
     reduce (stride 34) -- then an ACT in-place scale by 1/128.

The +32-row halo lives in G-space: partition p's G rows [784, 816) equal
partition p+1's G rows [0, 32), filled by one tiny partition-shifted
SBUF->SBUF DMA (537 KB/batch) after chunk 0's G-pass; partition 127's
tail stays zero (memset once).  Chunk order per batch is 0, 13, 12, .., 1
so the shift source exists before the first diag needs the tail.

Output stores, and the shift, issue on the ACT HWDGE ring (nc.scalar) so
they never head-of-line-block the loads on the sync ring.
"""

import numpy as np

B, T, S = 16, 100000, 128
NCORES = 8
BC = B // NCORES          # batches per core
LS = 784                  # stripe rows per partition (128*784 = 100352 >= T)
TP = 128 * LS             # padded rows per batch
HALO = 32                 # max delay
LC = 56                   # chunk rows
NCH = LS // LC            # 14 chunks per batch
G_W = 33                  # group-sum entries per row
SCALE = 1.0 / S

_cache = {}


def _build():
    import concourse.bass as bass
    import concourse.tile as tile
    from concourse import bacc, mybir

    f32 = mybir.dt.float32
    nc = bacc.Bacc("TRN2", target_bir_lowering=False, debug=False, num_devices=1)
    x = nc.dram_tensor("x", [BC * TP * S], f32, kind="ExternalInput")
    y = nc.dram_tensor("y", [BC * TP], f32, kind="ExternalOutput")

    def dram_ap(base_elem, rows):
        # [128 partitions (stripe-major), rows*S contiguous elems each]
        return bass.AP(x.ap().tensor, base_elem, [[LS * S, 128], [1, rows * S]])

    def sub_ap(t, off, dims):
        # custom AP into a tile: keep its partition dim, replace free dims
        return bass.AP(t.tensor, t.offset + off, [list(t.ap[0])] + dims)

    X = mybir.AxisListType.X

    with tile.TileContext(nc) as tc:
        from contextlib import ExitStack

        with ExitStack() as ctx:
            ring_pool = ctx.enter_context(tc.tile_pool(name="ring", bufs=1))
            g_pool = ctx.enter_context(tc.tile_pool(name="g", bufs=1))
            o_pool = ctx.enter_context(tc.tile_pool(name="o", bufs=2))

            ring = ring_pool.tile([128, 3 * LC * S], f32)
            G = g_pool.tile([128, (LS + HALO) * G_W], f32)

            # partition 127's G tail (rows past the whole batch) stays 0
            nc.vector.memset(G[:, LS * G_W : (LS + HALO) * G_W], 0.0)

            load_i = 0

            def load(b, c):
                nonlocal load_i
                slot = load_i % 3
                load_i += 1
                nc.sync.dma_start(
                    sub_ap(ring, slot * LC * S, [[1, LC * S]]),
                    dram_ap(b * TP * S + c * LC * S, LC),
                )
                return slot

            def g_pass(c, slot):
                base = slot * LC * S
                gbase = c * LC * G_W
                # groups k=1..31: sensors 4k-1..4k+2 -> G[t][1..31]
                nc.vector.reduce_sum(
                    sub_ap(G, gbase + 1, [[G_W, LC], [1, 31]]),
                    sub_ap(ring, base + 3, [[S, LC], [4, 31], [1, 4]]),
                    axis=X,
                )
                # group d=0: sensors 0..2 -> G[t][0]
                nc.vector.reduce_sum(
                    sub_ap(G, gbase, [[G_W, LC]]),
                    sub_ap(ring, base, [[S, LC], [1, 3]]),
                    axis=X,
                )
                # sensor 127 (d=32) -> G[t][32], on ACT
                nc.scalar.copy(
                    sub_ap(G, gbase + 32, [[G_W, LC]]),
                    sub_ap(ring, base + 127, [[S, LC]]),
                )

            def diag(c, out_sb):
                o = out_sb[:, c * LC : (c + 1) * LC]
                nc.vector.reduce_sum(
                    o,
                    sub_ap(G, c * LC * G_W, [[G_W, LC], [G_W + 1, G_W]]),
                    axis=X,
                )
                nc.scalar.mul(o, o, SCALE)

            for b in range(BC):
                out_sb = o_pool.tile([128, LS], f32, tag="out_sb")

                # chunk 0 first: its G head feeds the partition-shifted tail
                slot = load(b, 0)
                g_pass(0, slot)
                nc.scalar.dma_start(
                    G[0:127, LS * G_W : (LS + HALO) * G_W],
                    G[1:128, 0 : HALO * G_W],
                )

                for c in range(NCH - 1, 0, -1):
                    slot = load(b, c)
                    g_pass(c, slot)
                    diag(c, out_sb)
                    if c == NCH // 2:
                        # top half of the batch output is complete
                        nc.scalar.dma_start(
                            bass.AP(
                                y.ap().tensor,
                                b * TP + c * LC,
                                [[LS, 128], [1, LS - c * LC]],
                            ),
                            out_sb[:, c * LC : LS],
                        )
                diag(0, out_sb)
                nc.scalar.dma_start(
                    bass.AP(
                        y.ap().tensor,
                        b * TP,
                        [[LS, 128], [1, (NCH // 2) * LC]],
                    ),
                    out_sb[:, 0 : (NCH // 2) * LC],
                )

    nc.compile()
    return nc


def _get_nc():
    if "nc" not in _cache:
        _cache["nc"] = _build()
    return _cache["nc"]


def kernel(microphone_array: np.ndarray) -> np.ndarray:
    from concourse.bass_utils import run_bass_kernel_spmd

    x = np.asarray(microphone_array, dtype=np.float32)
    assert x.shape == (B, T, S)
    nc = _get_nc()

    in_maps = []
    for c in range(NCORES):
        shard = np.zeros((BC, TP, S), dtype=np.float32)
        shard[:, :T] = x[c * BC : (c + 1) * BC]
        in_maps.append({"x": shard.reshape(-1)})

    res = _cache["res"] = run_bass_kernel_spmd(
        nc, in_maps, core_ids=list(range(NCORES)), trace=_cache.get("trace", False)
    )

    out = np.empty((B, T), dtype=np.float32)
    for c in range(NCORES):
        out[c * BC : (c + 1) * BC] = res.results[c]["y"].reshape(BC, TP)[:, :T]
    return out
